# revision 1
# baseline (speedup 1.0000x reference)
"""Jacobi->Cartesian transform kernel for Trainium2 (8 NeuronCores, SPMD).

Math: for each batch b the reference computes x = inv(A(m_b)) @ r for every
trajectory step, where A is the Cartesian->Jacobi matrix. inv(A) has a closed
form: with M_i = cumsum(m)_i, c_i = m_i / M_i, s_i = c_i * r_i:

    x_k = r_k + s_0 - S_k,   S_k = sum_{i>=k} s_i   (suffix sum over particles)

which holds for all k (including k=0, since c_0 == 1 -> s_0 = r_0).

Device program per (batch, tensor) unit, in the natural [t, (k,d)] layout
(partition = t-block, free = (t_in, k, d)):
    S'[15] = c_15*r[15] - r[0]              (scalar_tensor_tensor, FD=96)
    S'[k]  = c_k *r[k]  + S'[k+1]  k=14..0  (scalar_tensor_tensor, FD=96)
    x      = r - S'                         (tensor_sub, FD=1536)
No transposes, no PE, no PSUM; DMA-bound by design.

Sharding: pure data parallelism, 16 batches per core across 8 cores.
"""

import numpy as np

import concourse.bacc as bacc
import concourse.mybir as mybir
from concourse.tile import TileContext
from concourse.bass_utils import run_bass_kernel_spmd

B, T, N, D = 128, 4096, 16, 3
N_CORES = 8
BPC = B // N_CORES          # batches per core
P = 128                     # partitions
TI = T // P                 # 32 t's per partition
FREE = TI * N * D           # 1536 free elements per partition

_CACHE = {}


def build_bass():
    if "nc" in _CACHE:
        return _CACHE["nc"]
    nc = bacc.Bacc(
        "TRN2",
        target_bir_lowering=False,
        debug=False,
        enable_asserts=False,
        num_devices=N_CORES,
    )
    f32 = mybir.dt.float32
    qj = nc.dram_tensor("qj", [BPC, T, N, D], f32, kind="ExternalInput").ap()
    vj = nc.dram_tensor("vj", [BPC, T, N, D], f32, kind="ExternalInput").ap()
    coef = nc.dram_tensor("coef", [P, BPC * N], f32, kind="ExternalInput").ap()
    q = nc.dram_tensor("q", [BPC, T, N, D], f32, kind="ExternalOutput").ap()
    v = nc.dram_tensor("v", [BPC, T, N, D], f32, kind="ExternalOutput").ap()

    PR = 2  # batches per DMA/compute unit
    with TileContext(nc) as tc:
        with (
            tc.tile_pool(name="coefp", bufs=1) as coefp,
            tc.tile_pool(name="rp", bufs=4) as rp,
            tc.tile_pool(name="sp", bufs=3) as sp,
        ):
            coef_sb = coefp.tile([P, BPC * N], f32)
            nc.sync.dma_start(out=coef_sb[:], in_=coef)

            for b0 in range(0, BPC, PR):
                for src, dst in ((qj, q), (vj, v)):
                    r = rp.tile([P, PR * FREE], f32)
                    r5 = r[:].rearrange("p (b ti k d) -> p b ti k d", b=PR, k=N, d=D)
                    nc.sync.dma_start(
                        out=r5,
                        in_=src[b0 : b0 + PR].rearrange(
                            "b (p ti) k d -> p b ti k d", p=P
                        ),
                    )
                    s = sp.tile([P, PR * FREE], f32)
                    s5 = s[:].rearrange("p (b ti k d) -> p b ti k d", b=PR, k=N, d=D)

                    for bi in range(PR):
                        b = b0 + bi

                        def ck(k, b=b):
                            return coef_sb[:, b * N + k : b * N + k + 1]

                        # S'[15] = c15*r[15] - r[0]
                        nc.vector.scalar_tensor_tensor(
                            out=s5[:, bi : bi + 1, :, N - 1 : N, :],
                            in0=r5[:, bi : bi + 1, :, N - 1 : N, :],
                            scalar=ck(N - 1),
                            in1=r5[:, bi : bi + 1, :, 0:1, :],
                            op0=mybir.AluOpType.mult,
                            op1=mybir.AluOpType.subtract,
                        )
                        # S'[k] = ck*r[k] + S'[k+1]
                        for k in range(N - 2, -1, -1):
                            nc.vector.scalar_tensor_tensor(
                                out=s5[:, bi : bi + 1, :, k : k + 1, :],
                                in0=r5[:, bi : bi + 1, :, k : k + 1, :],
                                scalar=ck(k),
                                in1=s5[:, bi : bi + 1, :, k + 1 : k + 2, :],
                                op0=mybir.AluOpType.mult,
                                op1=mybir.AluOpType.add,
                            )
                    # x = r - S'  (in place into r)
                    nc.vector.tensor_sub(out=r[:], in0=r[:], in1=s[:])
                    nc.sync.dma_start(
                        out=dst[b0 : b0 + PR].rearrange(
                            "b (p ti) k d -> p b ti k d", p=P
                        ),
                        in_=r[:].rearrange("p (b ti k d) -> p b ti k d", b=PR, k=N, d=D),
                    )
    nc.compile()
    _CACHE["nc"] = nc
    return nc


def make_in_maps(m, qj, vj):
    m = np.asarray(m, dtype=np.float32)
    qj = np.asarray(qj, dtype=np.float32)
    vj = np.asarray(vj, dtype=np.float32)
    M = np.cumsum(m.astype(np.float64), axis=-1)
    c = (m.astype(np.float64) / M).astype(np.float32)  # [B, N]
    in_maps = []
    for core in range(N_CORES):
        bs = slice(core * BPC, (core + 1) * BPC)
        coef_rep = np.ascontiguousarray(
            np.broadcast_to(c[bs].reshape(1, BPC * N), (P, BPC * N))
        )
        in_maps.append(
            {
                "qj": np.ascontiguousarray(qj[bs]),
                "vj": np.ascontiguousarray(vj[bs]),
                "coef": coef_rep,
            }
        )
    return in_maps


def kernel(m, qj, vj):
    nc = build_bass()
    in_maps = make_in_maps(m, qj, vj)
    res = run_bass_kernel_spmd(nc, in_maps, core_ids=list(range(N_CORES)))
    q = np.concatenate([res.results[i]["q"] for i in range(N_CORES)], axis=0)
    v = np.concatenate([res.results[i]["v"] for i in range(N_CORES)], axis=0)
    return q, v



# revision 3
# speedup vs baseline: 1.6769x; 1.6769x over previous
"""Jacobi->Cartesian transform kernel for Trainium2 (8 NeuronCores, SPMD).

Math: for each batch b the reference computes x = inv(A(m_b)) @ r for every
trajectory step. inv(A) has a closed form: with M_i = cumsum(m)_i,
c_i = m_i / M_i (c_0 == 1 exactly), s_i = c_i * r_i:

    x_k = r_k - S'_k,   S'_15 = c_15 r_15 - r_0,  S'_k = c_k r_k + S'_{k+1}

Device design (per core):
  - All bulk IO in fp16 (tolerance 2e-2; fp16 pipeline gives ~1e-3), which
    halves HBM traffic vs f32: 25.2 MB/core -> ~70us DMA floor at 360 GB/s.
  - Partition layout p = (batch, t_block): 16 batches x 8 t-blocks = 128
    partitions, 512 t's each. The per-(batch,k) coefficient c_k is then a
    per-partition scalar, so one vector op per k covers ALL batches at once.
  - Scan on DVE as: 15 tensor_scalar products (4x fp16 mode, k=0 skipped
    since c_0=1 -> s_0 = r_0), then 16 tensor_tensor chain ops (2x mode).
  - Final x = r - S' tensor_sub split between DVE (2x) and Pool/GPSIMD by a
    per-chunk fraction so both engines stay under the DMA floor; later
    chunks give Pool less so the last output DMA isn't delayed.
  - in-DMAs issued from the SP queue, out-DMAs from the otherwise-idle
    Activation engine queue so output sem waits never block input prefetch.

Sharding: pure data parallelism, 16 batches per core across 8 cores.
Host side: f32<->f16 casts and pure-view reshapes; coefficients f32.
"""

import numpy as np

import concourse.bacc as bacc
import concourse.mybir as mybir
from concourse.tile import TileContext
from concourse.bass_utils import run_bass_kernel_spmd

B, T, N, D = 128, 4096, 16, 3
N_CORES = 8
BPC = B // N_CORES          # batches per core
P = 128                     # partitions
TBLK = P // BPC             # 8 t-blocks per batch
TB = T // TBLK              # 512 t's per partition
ND = N * D                  # 48
TC = 256                    # t's per chunk
NCHUNK = TB // TC           # 2 chunks per tensor
FREE = TC * ND              # 12288 free elements per partition per chunk

# fraction of each chunk's final sub done on Pool/GPSIMD, in emission order
# (q0, v0, q1, v1); tapered so the tail chunk's output isn't Pool-gated
POOL_FRAC = (0.70, 0.70, 0.50, 0.30)

_CACHE = {}


def build_bass():
    if "nc" in _CACHE:
        return _CACHE["nc"]
    nc = bacc.Bacc(
        "TRN2",
        target_bir_lowering=False,
        debug=False,
        enable_asserts=False,
        num_devices=N_CORES,
    )
    f32 = mybir.dt.float32
    f16 = mybir.dt.float16
    AL = mybir.AluOpType
    qj = nc.dram_tensor("qj", [P, TB, ND], f16, kind="ExternalInput").ap()
    vj = nc.dram_tensor("vj", [P, TB, ND], f16, kind="ExternalInput").ap()
    coef = nc.dram_tensor("coef", [P, N], f32, kind="ExternalInput").ap()
    q = nc.dram_tensor("q", [P, TB, ND], f16, kind="ExternalOutput").ap()
    v = nc.dram_tensor("v", [P, TB, ND], f16, kind="ExternalOutput").ap()

    with TileContext(nc) as tc:
        with (
            tc.tile_pool(name="coefp", bufs=1) as coefp,
            tc.tile_pool(name="rp", bufs=4) as rp,
            tc.tile_pool(name="sp", bufs=3) as sp,
        ):
            coef_sb = coefp.tile([P, N], f32)
            nc.sync.dma_start(out=coef_sb[:], in_=coef)

            unit = 0
            for c in range(NCHUNK):
                for src, dst in ((qj, q), (vj, v)):
                    r = rp.tile([P, FREE], f16)
                    r3 = r[:].rearrange("p (ti kd) -> p ti kd", kd=ND)
                    nc.sync.dma_start(
                        out=r3, in_=src[:, c * TC : (c + 1) * TC, :]
                    )
                    r5 = r[:].rearrange("p (ti k d) -> p ti k d", k=N, d=D)
                    s = sp.tile([P, FREE], f16)
                    s5 = s[:].rearrange("p (ti k d) -> p ti k d", k=N, d=D)

                    # products s~_k = c_k * r_k for k=1..15 (4x fp16 mode)
                    for k in range(1, N):
                        nc.vector.tensor_scalar(
                            out=s5[:, :, k : k + 1, :],
                            in0=r5[:, :, k : k + 1, :],
                            scalar1=coef_sb[:, k : k + 1],
                            scalar2=None,
                            op0=AL.mult,
                        )
                    # S'[15] = s~_15 - r_0
                    nc.vector.tensor_tensor(
                        out=s5[:, :, N - 1 : N, :],
                        in0=s5[:, :, N - 1 : N, :],
                        in1=r5[:, :, 0:1, :],
                        op=AL.subtract,
                    )
                    # S'[k] = s~_k + S'[k+1], k=14..1
                    for k in range(N - 2, 0, -1):
                        nc.vector.tensor_tensor(
                            out=s5[:, :, k : k + 1, :],
                            in0=s5[:, :, k : k + 1, :],
                            in1=s5[:, :, k + 1 : k + 2, :],
                            op=AL.add,
                        )
                    # S'[0] = r_0 + S'[1]   (s~_0 = r_0 since c_0 = 1)
                    nc.vector.tensor_tensor(
                        out=s5[:, :, 0:1, :],
                        in0=r5[:, :, 0:1, :],
                        in1=s5[:, :, 1:2, :],
                        op=AL.add,
                    )
                    # x = r - S' (in place into r), split DVE / Pool
                    split = int(round(POOL_FRAC[unit] * TC)) * ND
                    if split > 0:
                        nc.gpsimd.tensor_tensor(
                            out=r[:, :split],
                            in0=r[:, :split],
                            in1=s[:, :split],
                            op=AL.subtract,
                        )
                    if split < FREE:
                        nc.vector.tensor_tensor(
                            out=r[:, split:],
                            in0=r[:, split:],
                            in1=s[:, split:],
                            op=AL.subtract,
                        )
                    nc.scalar.dma_start(
                        out=dst[:, c * TC : (c + 1) * TC, :], in_=r3
                    )
                    unit += 1
    nc.compile()
    _CACHE["nc"] = nc
    return nc


def make_in_maps(m, qj, vj):
    m = np.asarray(m, dtype=np.float32)
    M = np.cumsum(m.astype(np.float64), axis=-1)
    c = (m.astype(np.float64) / M).astype(np.float32)  # [B, N]
    qj16 = np.asarray(qj, dtype=np.float16)
    vj16 = np.asarray(vj, dtype=np.float16)
    in_maps = []
    for core in range(N_CORES):
        bs = slice(core * BPC, (core + 1) * BPC)
        in_maps.append(
            {
                # [BPC, T, N, D] -> [P, TB, ND]: pure row-major reshape
                "qj": np.ascontiguousarray(qj16[bs]).reshape(P, TB, ND),
                "vj": np.ascontiguousarray(vj16[bs]).reshape(P, TB, ND),
                "coef": np.ascontiguousarray(np.repeat(c[bs], TBLK, axis=0)),
            }
        )
    return in_maps


def kernel(m, qj, vj):
    nc = build_bass()
    in_maps = make_in_maps(m, qj, vj)
    res = run_bass_kernel_spmd(nc, in_maps, core_ids=list(range(N_CORES)))
    qs, vs = [], []
    for i in range(N_CORES):
        qs.append(
            res.results[i]["q"].reshape(BPC, T, N, D).astype(np.float32)
        )
        vs.append(
            res.results[i]["v"].reshape(BPC, T, N, D).astype(np.float32)
        )
    return np.concatenate(qs, axis=0), np.concatenate(vs, axis=0)


# revision 15
# speedup vs baseline: 2.1282x; 1.2691x over previous
"""Jacobi->Cartesian transform kernel for Trainium2 (8 NeuronCores, SPMD).

Math: for each batch b the reference computes x = inv(A(m_b)) @ r for every
trajectory step. inv(A) has a closed form: with M_i = cumsum(m)_i,
c_i = m_i / M_i (c_0 == 1 exactly), s_i = c_i * r_i:

    x_k = r_k - S'_k,   S'_15 = c_15 r_15 - r_0,  S'_k = c_k r_k + S'_{k+1}

Device design (per core):
  - All bulk IO in fp16 (tolerance 2e-2; fp16 pipeline gives ~1.6e-3), which
    halves HBM traffic vs f32: 25.2 MB/core -> ~70us DMA floor at 360 GB/s.
  - Partition layout p = (batch, t_block): 16 batches x 8 t-blocks = 128
    partitions, 512 t's each. The per-(batch,k) coefficient c_k is then a
    per-partition scalar, so one op per k covers ALL batches at once.
  - Per chunk: products s~_k = c_k * r_k on the Activation engine
    (activation Copy with per-partition scale AP) and/or DVE tensor_scalar
    (4x fp16 mode), emitted descending in k; the 16-op suffix chain on DVE
    tensor_tensor (2x); the final x = r - S' sub split DVE / Pool(GPSIMD).
    k=0 products skipped (c_0 = 1 -> s_0 = r_0).
  - ALL in-DMAs are emitted (and their r tiles allocated) before any
    compute so the SP sequencer never parks an input behind an output's
    sem wait; outputs issue from SP after, coef from the Act queue.
  - First chunk is small so compute starts early; sizes taper at the end
    so the last output's compute tail fits under the DMA stream.

Sharding: pure data parallelism, 16 batches per core across 8 cores.
Host side: f32<->f16 casts and pure-view reshapes; coefficients f32.
"""

import contextlib

import numpy as np

import concourse.bacc as bacc
import concourse.mybir as mybir
from concourse.tile import TileContext
from concourse.bass_utils import run_bass_kernel_spmd

B, T, N, D = 128, 4096, 16, 3
N_CORES = 8
BPC = B // N_CORES          # batches per core
P = 128                     # partitions
TBLK = P // BPC             # 8 t-blocks per batch
TB = T // TBLK              # 512 t's per partition
ND = N * D                  # 48

# per-tensor chunk sizes along the per-partition t axis (must sum to TB)
SIZES = (64, 176, 144, 128)
# per emitted unit (q0,v0,q1,v1,...): products k<=KA on Act, k>KA on DVE
KA = (0, 0, 12, 15, 15, 15, 9, 12)
# Pool fraction of each unit's final sub
BETA = (0.6, 0.85, 0.5, 0.5, 0.5, 0.3, 0.25, 0.2)

_CACHE = {}


def build_bass(sizes=SIZES, ka=KA, beta=BETA, spb=2, cache=True):
    if cache and "nc" in _CACHE:
        return _CACHE["nc"]
    assert sum(sizes) == TB
    nc = bacc.Bacc(
        "TRN2",
        target_bir_lowering=False,
        debug=False,
        enable_asserts=False,
        num_devices=N_CORES,
    )
    f32 = mybir.dt.float32
    f16 = mybir.dt.float16
    AL = mybir.AluOpType
    qj = nc.dram_tensor("qj", [P, TB, ND], f16, kind="ExternalInput").ap()
    vj = nc.dram_tensor("vj", [P, TB, ND], f16, kind="ExternalInput").ap()
    coef = nc.dram_tensor("coef", [P, N], f32, kind="ExternalInput").ap()
    q = nc.dram_tensor("q", [P, TB, ND], f16, kind="ExternalOutput").ap()
    v = nc.dram_tensor("v", [P, TB, ND], f16, kind="ExternalOutput").ap()

    # unit list: (src, dst, t0, tc_sz)
    units = []
    t0 = 0
    for tc_sz in sizes:
        for src, dst in ((qj, q), (vj, v)):
            units.append((src, dst, t0, tc_sz))
        t0 += tc_sz

    uniq = sorted(set(sizes))
    with TileContext(nc) as tc, contextlib.ExitStack() as stack:
        coefp = stack.enter_context(tc.tile_pool(name="coefp", bufs=1))
        spools, rpools = {}, {}
        for sz in uniq:
            n_units = 2 * sizes.count(sz)
            spools[sz] = stack.enter_context(
                tc.tile_pool(name=f"sp{sz}", bufs=min(n_units, spb))
            )
            # every r tile lives for the whole program: allocate all up front
            rpools[sz] = stack.enter_context(
                tc.tile_pool(name=f"rp{sz}", bufs=n_units)
            )

        coef_sb = coefp.tile([P, N], f32)
        nc.scalar.dma_start(out=coef_sb[:], in_=coef)

        rtiles = []
        for src, dst, t0, tc_sz in units:
            r = rpools[tc_sz].tile([P, tc_sz * ND], f16)
            r3 = r[:].rearrange("p (ti kd) -> p ti kd", kd=ND)
            nc.sync.dma_start(out=r3, in_=src[:, t0 : t0 + tc_sz, :])
            rtiles.append((r, r3))

        for unit, (src, dst, t0, tc_sz) in enumerate(units):
            r, r3 = rtiles[unit]
            free = tc_sz * ND
            r5 = r[:].rearrange("p (ti k d) -> p ti k d", k=N, d=D)
            s = spools[tc_sz].tile([P, free], f16)
            s5 = s[:].rearrange("p (ti k d) -> p ti k d", k=N, d=D)

            # products s~_k = c_k * r_k, emitted descending (chain order);
            # Act gets the low k's (needed last), DVE the high k's
            for k in range(N - 1, 0, -1):
                if k <= ka[unit]:
                    nc.scalar.mul(
                        out=s5[:, :, k : k + 1, :],
                        in_=r5[:, :, k : k + 1, :],
                        mul=coef_sb[:, k : k + 1],
                    )
                else:
                    nc.vector.tensor_scalar(
                        out=s5[:, :, k : k + 1, :],
                        in0=r5[:, :, k : k + 1, :],
                        scalar1=coef_sb[:, k : k + 1],
                        scalar2=None,
                        op0=AL.mult,
                    )
            # S'[15] = s~_15 - r_0
            nc.vector.tensor_tensor(
                out=s5[:, :, N - 1 : N, :],
                in0=s5[:, :, N - 1 : N, :],
                in1=r5[:, :, 0:1, :],
                op=AL.subtract,
            )
            # S'[k] = s~_k + S'[k+1], k=14..1
            for k in range(N - 2, 0, -1):
                nc.vector.tensor_tensor(
                    out=s5[:, :, k : k + 1, :],
                    in0=s5[:, :, k : k + 1, :],
                    in1=s5[:, :, k + 1 : k + 2, :],
                    op=AL.add,
                )
            # S'[0] = r_0 + S'[1]
            nc.vector.tensor_tensor(
                out=s5[:, :, 0:1, :],
                in0=r5[:, :, 0:1, :],
                in1=s5[:, :, 1:2, :],
                op=AL.add,
            )
            # x = r - S' (in place into r), split Pool / DVE
            split = int(round(beta[unit] * tc_sz)) * ND
            if split > 0:
                nc.gpsimd.tensor_tensor(
                    out=r[:, :split],
                    in0=r[:, :split],
                    in1=s[:, :split],
                    op=AL.subtract,
                )
            if split < free:
                nc.vector.tensor_tensor(
                    out=r[:, split:],
                    in0=r[:, split:],
                    in1=s[:, split:],
                    op=AL.subtract,
                )
            nc.sync.dma_start(out=dst[:, t0 : t0 + tc_sz, :], in_=r3)
    nc.compile()
    if cache:
        _CACHE["nc"] = nc
    return nc


def make_in_maps(m, qj, vj):
    m = np.asarray(m, dtype=np.float32)
    M = np.cumsum(m.astype(np.float64), axis=-1)
    c = (m.astype(np.float64) / M).astype(np.float32)  # [B, N]
    qj16 = np.asarray(qj, dtype=np.float16)
    vj16 = np.asarray(vj, dtype=np.float16)
    in_maps = []
    for core in range(N_CORES):
        bs = slice(core * BPC, (core + 1) * BPC)
        in_maps.append(
            {
                # [BPC, T, N, D] -> [P, TB, ND]: pure row-major reshape
                "qj": np.ascontiguousarray(qj16[bs]).reshape(P, TB, ND),
                "vj": np.ascontiguousarray(vj16[bs]).reshape(P, TB, ND),
                "coef": np.ascontiguousarray(np.repeat(c[bs], TBLK, axis=0)),
            }
        )
    return in_maps


def kernel(m, qj, vj):
    nc = build_bass()
    in_maps = make_in_maps(m, qj, vj)
    res = run_bass_kernel_spmd(nc, in_maps, core_ids=list(range(N_CORES)))
    qs, vs = [], []
    for i in range(N_CORES):
        qs.append(
            res.results[i]["q"].reshape(BPC, T, N, D).astype(np.float32)
        )
        vs.append(
            res.results[i]["v"].reshape(BPC, T, N, D).astype(np.float32)
        )
    return np.concatenate(qs, axis=0), np.concatenate(vs, axis=0)
